# revision 34
# baseline (speedup 1.0000x reference)
"""Distributed causal multi-head attention for TRN2 (8 NeuronCores).

Problem: B=2, T=2048, D=1024, H=16 heads (head_dim 64), causal MHA:
  q,k,v = x@W{q,k,v}+b, q *= dh**-0.5, o = softmax(mask(q k^T)) v, out = o@Wp + bp

Sharding: 8-way tensor parallel over heads for attention; token-parallel
for the output projection.  Core r handles BOTH batches and heads
{2r, 2r+1} through attention, then the cores redistribute oT with small
AllToAlls -- one per chunk for chunks 0-2 (fires as soon as that chunk's
attention finishes for both batches) and one per batch for chunk 3 (so
batch 0's collective overlaps batch 1's attention and only a 128KB
collective is tail-exposed).  After the swap core r owns 64-token slices
of every chunk ({c*512 + 64r .. +64} per batch) and projects them
against the FULL Wp.  This moves 8x less data than an AllGather scheme
(each core receives 0.9MB instead of 7MB).  Measured collective
behavior this design works around: each collective has ~10-20us of
latency dominated by cross-core skew, collectives serialize on the CC
core, and DMAs issued after a collective in program order wait for its
completion.  Per core:
  - QKV projections in fp16 on TensorE (q/k produced transposed [hd, t],
    v produced natural [t, hd] with an appended ones-column)
  - scores computed transposed [keys, q] (K=64 contraction, two heads
    packed into the 128x128 PE array via row tiling, both writing halves
    of one 2-bank PSUM tile); diagonal key tiles only compute the valid
    (q >= key) column range; ONE fused exp per key tile on ScalarE;
    causal handled by key-tile skipping + a post-exp 0/1 mask multiply
    on the diagonal blocks (stale below-diagonal PSUM is exp'd but
    never read by AV)
  - AV uses exp-weights as the stationary operand -> o natural [q, hd]
    with per-partition row sums for free (ones column of v); normalize
    with a per-partition reciprocal; AV interleaves with scores at lag 1
    so the PE never waits long on ScalarE
  - o is PE-transposed locally (cheap) so the AllToAll carries oT and
    the output projection needs no DMA transposes
  - output projection: group 0 (chunks 0+1) fills the attention
    wind-down; (1,0) runs under the last collective's flight; only
    (1,1) pays one exposed collective latency.  oT tiles arrive in one
    SBUF tile via two DMAs (dma_start costs ~0.6us of sequencer time);
    PSUM->SBUF casts run on ScalarE, idle after the last exp.
Host side only shards/converts inputs, concatenates token slices, and
adds the bias terms that are mathematically output-constant (bv@Wp + bp;
bk cancels in softmax; bq is applied on device).
"""

import os
import numpy as np

B, T, D, H = 2, 2048, 1024, 16
DH = 64
NCORES = 8
HPC = H // NCORES      # heads per core = 2
CD = HPC * DH          # per-core head-dim = 128
P = 128
NCH = 4                # T chunks
CHUNK = T // NCH       # 512
KT = T // P            # 16 key tiles
KD = D // P            # 8 contraction tiles for the projections
NG = 2                 # chunk-pair groups for the AllToAll

_CACHE = {}

# Results of the last device run (for test harnesses): BassKernelResults
LAST_RESULT = None


def _build_nc():
    import concourse.bass as bass
    import concourse.mybir as mybir
    import concourse.tile as tile
    from concourse import bacc
    from contextlib import ExitStack

    fp = mybir.dt.float16
    f32 = mybir.dt.float32
    AF = mybir.ActivationFunctionType

    nc = bacc.Bacc("TRN2", target_bir_lowering=False, debug=False,
                   num_devices=NCORES)

    xT = nc.dram_tensor("xT", [D, B, T], fp, kind="ExternalInput").ap()
    wq = nc.dram_tensor("wq", [D, CD], fp, kind="ExternalInput").ap()
    wk = nc.dram_tensor("wk", [D, CD], fp, kind="ExternalInput").ap()
    wv = nc.dram_tensor("wv", [D, CD], fp, kind="ExternalInput").ap()
    wp = nc.dram_tensor("wp", [D, D], fp, kind="ExternalInput").ap()
    bqp = nc.dram_tensor("bqp", [P, 1], f32, kind="ExternalInput").ap()
    maskf = nc.dram_tensor("maskf", [P, P], fp, kind="ExternalInput").ap()
    ident = nc.dram_tensor("ident", [P, P], fp, kind="ExternalInput").ap()
    # core r owns token rows [g*1024 + 128r, +128) of each batch
    out = nc.dram_tensor("out", [NG, B, P, D], fp, kind="ExternalOutput").ap()

    # AllToAll staging, one collective per chunk (fires as soon as that
    # chunk's attention output is staged for both batches; only the last
    # chunk's 256KB collective is exposed at the tail).  in[j] = my oT
    # for the 64 tokens of this chunk that dest j owns (both batches);
    # out[s] = core s's oT (its 128 head-dims) for MY 64 tokens.
    a2ain = [nc.dram_tensor(f"a2ain_{c}", [NCORES, B, CD, 64], fp).ap()
             for c in range(NCH - 1)]
    a2aout = [nc.dram_tensor(f"a2aout_{c}", [NCORES, B, CD, 64], fp).ap()
              for c in range(NCH - 1)]
    # chunk 3 ships per batch: batch 0's collective overlaps batch 1's
    # attention, so only batch 1's 128KB collective is tail-exposed
    a2ain3 = [nc.dram_tensor(f"a2ain3_{b}", [NCORES, CD, 64], fp).ap()
              for b in range(B)]
    a2aout3 = [nc.dram_tensor(f"a2aout3_{b}", [NCORES, CD, 64], fp).ap()
               for b in range(B)]
    warm_in = nc.dram_tensor("warm_in", [NCORES, P], fp).ap()
    warm_out = nc.dram_tensor("warm_out", [NCORES, P], fp).ap()

    RG = [[0, 1, 2, 3, 4, 5, 6, 7]]

    with tile.TileContext(nc, num_cores=NCORES) as tc, ExitStack() as ctx:
        const = ctx.enter_context(tc.tile_pool(name="const", bufs=1))
        work = ctx.enter_context(tc.tile_pool(name="work", bufs=3))
        expp = ctx.enter_context(tc.tile_pool(name="expp", bufs=18))
        otkp = ctx.enter_context(tc.tile_pool(name="otkp", bufs=18))
        osbp = ctx.enter_context(tc.tile_pool(name="osbp", bufs=8))
        psum = ctx.enter_context(tc.tile_pool(name="psum", bufs=2,
                                              space="PSUM"))

        # ---- persistent SBUF ----
        xT_sb = const.tile([P, KD, B, T], fp)        # 64 KB/p
        wq_sb = const.tile([P, KD, CD], fp)
        wk_sb = const.tile([P, KD, CD], fp)
        wv_sb = const.tile([P, KD, CD], fp)
        wp_sb = const.tile([P, KD, D], fp)           # FULL Wp, 16 KB/p
        bq_sb = const.tile([P, 1], f32)
        mask_sb = const.tile([P, P], fp)             # 0/1 lower triangle
        ident_sb = const.tile([P, P], fp)
        qT_sb = const.tile([P, B, T], fp)            # 2 heads stacked
        kT_sb = const.tile([P, B, T], fp)
        v_sb = const.tile([P, KT, B, HPC, DH + 1], fp)

        # DMA issue order = approximate arrival order: the first q-unit
        # gate (wq + x chunk0 batch0) lands ~4us in; wp (2MB, needed at
        # ~half-kernel) goes last.
        # Head DMAs: each dma_start costs ~0.6us of its sequencer's
        # time, so the first-needed loads are spread across the (idle)
        # sync, scalar, and vector sequencers to overlap issue latency.
        # Chunk-0 batch-0 x is split per k-tile so the first matmul can
        # fire after 128KB instead of 1MB.
        xT_r = xT.rearrange("(k p) b t -> p k b t", p=P)
        nc.sync.dma_start(wq_sb[:], wq.rearrange("(k p) c -> p k c", p=P))
        for k in range(0, KD, 2):
            nc.scalar.dma_start(xT_sb[:, k, 0, 0:512],
                                xT_r[:, k, 0, 0:512])
            nc.gpsimd.dma_start(xT_sb[:, k + 1, 0, 0:512],
                                xT_r[:, k + 1, 0, 0:512])
        nc.sync.dma_start(wk_sb[:], wk.rearrange("(k p) c -> p k c", p=P))
        nc.sync.dma_start(bq_sb[:], bqp)
        nc.sync.dma_start(xT_sb[:, :, 1, 0:512], xT_r[:, :, 1, 0:512])
        nc.sync.dma_start(wv_sb[:], wv.rearrange("(k p) c -> p k c", p=P))
        nc.sync.dma_start(mask_sb[:], maskf)
        nc.sync.dma_start(ident_sb[:], ident)
        nc.vector.memset(v_sb[:, :, :, :, DH:DH + 1], 1.0)
        for t4 in range(1, NCH):
            for b in range(B):
                nc.sync.dma_start(
                    xT_sb[:, :, b, t4 * 512:(t4 + 1) * 512],
                    xT_r[:, :, b, t4 * 512:(t4 + 1) * 512])
        nc.sync.dma_start(wp_sb[:], wp.rearrange("(k p) c -> p k c", p=P))
        # tiny warmup collective: absorbs the first-collective latency
        # anomaly while the input DMAs stream
        nc.gpsimd.collective_compute(
            "AllToAll", bass.mybir.AluOpType.bypass,
            replica_groups=RG, ins=[warm_in], outs=[warm_out])

        def qkv_units(t4):
            """Projection work for T-chunk t4 as a list of closures, so
            it can be drip-fed into the attention k-loop (fills the PE
            while ScalarE paces the exp pipeline)."""
            units = []

            psqk_box = {}

            def q_unit(b):
                psqk = psum.tile([P, 1024], f32, tag="big", bufs=3,
                                 name=f"psqk_{t4}_{b}")
                psqk_box[b] = psqk
                for k in range(KD):
                    nc.tensor.matmul(
                        psqk[:, 0:512], wq_sb[:, k, :],
                        xT_sb[:, k, b, t4 * 512:(t4 + 1) * 512],
                        start=(k == 0), stop=(k == KD - 1))
                nc.vector.tensor_scalar_add(
                    qT_sb[:, b, t4 * 512:(t4 + 1) * 512], psqk[:, 0:512],
                    bq_sb[:, 0:1])

            def k_unit(b):
                psqk = psqk_box[b]
                for k in range(KD):
                    nc.tensor.matmul(
                        psqk[:, 512:1024], wk_sb[:, k, :],
                        xT_sb[:, k, b, t4 * 512:(t4 + 1) * 512],
                        start=(k == 0), stop=(k == KD - 1))
                nc.vector.tensor_copy(
                    kT_sb[:, b, t4 * 512:(t4 + 1) * 512], psqk[:, 512:1024])

            def v_unit(b, tt):
                psv = psum.tile([P, 256], f32, tag="big", bufs=3,
                                name=f"psv_{tt}_{b}")
                for k in range(KD):
                    nc.tensor.matmul(
                        psv[:, :CD], xT_sb[:, k, b, tt * P:(tt + 1) * P],
                        wv_sb[:, k, :], start=(k == 0),
                        stop=(k == KD - 1))
                nc.vector.tensor_copy(
                    out=v_sb[:, tt, b, :, 0:DH],
                    in_=psv[:, :CD].rearrange("p (h d) -> p h d", h=HPC))

            for b in range(B):
                units.append((t4, b, lambda b=b: q_unit(b)))
                units.append((t4, b, lambda b=b: k_unit(b)))
                for tt in range(4 * t4, 4 * t4 + 4):
                    units.append((t4, b, lambda b=b, tt=tt: v_unit(b, tt)))
            return units

        def attention_batch(c, b, filler=None):
            """Causal attention for q-chunk c, batch b (2 heads packed).

            scores for both heads go into one [128,1024] PSUM tile
            (row-packed K=64 matmuls -> halves), one fused exp per key
            tile, AV interleaved with lag 1.  AV accumulates all four
            q-subtiles of each head in one PSUM bank (4 interleaved
            accumulation groups as column ranges)."""
            nkt = 4 * (c + 1)
            exp_tiles = {}
            pso = {}
            for hh in range(2):
                pso[hh] = psum.tile([P, 4, DH + 1], f32, tag="o",
                                    name=f"pso_{c}_{b}_{hh}")

            def do_scores(k):
                ps_s = psum.tile([P, 1024], f32, tag="big", bufs=3,
                                 name=f"ps_{c}_{b}_{k}")
                j = k - 4 * c
                # diagonal key tiles: only the q >= key column range is
                # ever read downstream -> skip the dead columns
                q0 = max(j, 0) * P
                for hh in range(2):
                    lo, hi = hh * DH, (hh + 1) * DH
                    nc.tensor.matmul(
                        ps_s[:, hh * 512 + q0:(hh + 1) * 512],
                        kT_sb[lo:hi, b, k * P:(k + 1) * P],
                        qT_sb[lo:hi, b, c * 512 + q0:(c + 1) * 512],
                        start=True, stop=True)
                e = expp.tile([P, 1024], fp, tag="expT",
                              name=f"expT_{c}_{b}_{k}")
                if j >= 2:
                    # deep diagonal tile: skip exp on the (never-read)
                    # below-diagonal columns - ScalarE is the pacer
                    for hh in range(2):
                        lo = hh * 512 + j * P
                        hi = (hh + 1) * 512
                        nc.scalar.activation(e[:, lo:hi], ps_s[:, lo:hi],
                                             AF.Exp)
                else:
                    # j==1 reads 128 stale PSUM cols per head; exp of
                    # stale data is never read by AV (s < j skipped)
                    nc.scalar.activation(e[:], ps_s[:], AF.Exp)
                if j >= 0:
                    blks = e[:].rearrange("p (hh q) -> p hh q", hh=2)[
                        :, :, j * P:(j + 1) * P]
                    nc.vector.tensor_mul(
                        blks, blks,
                        mask_sb[:, None, :].to_broadcast([P, 2, P]))
                exp_tiles[k] = e

            def do_av(k):
                # pso[hh] holds 4 interleaved accumulation groups in one
                # PSUM bank; only the first write of the bank (k==0,s==0)
                # may set start (bank-wide has_written clear).  For diag
                # key tiles (k>0) the mask-dependent s==j block goes last
                # so the other AV matmuls never queue behind the DVE mask.
                j = k - 4 * c
                order = list(range(4))
                if k > 0 and 0 <= j < 4:
                    order = [s for s in order if s != j] + [j]
                for hh in range(2):
                    for s in order:
                        if k <= 4 * c + s:
                            nc.tensor.matmul(
                                pso[hh][:, s, :],
                                exp_tiles[k][:, hh * 512 + s * P:
                                             hh * 512 + (s + 1) * P],
                                v_sb[:, k, b, hh, :],
                                start=(k == 0 and s == 0),
                                stop=(k == 4 * c + s),
                                skip_group_check=True)

            for k in range(nkt + 1):
                if k < nkt:
                    do_scores(k)
                if k > 0:
                    do_av(k - 1)
                if filler is not None and k >= 2 and (c == 0 or
                                                     k < nkt - 1):
                    # no fills near the end of the loop (except chunk 0,
                    # which feeds no collective): the collective-critical
                    # finish work must not queue behind drip units
                    filler()
            return pso

        def finish_batch(c, b, pso):
            """normalize -> PE transpose -> AllToAll staging buffer.
            The staging DMAs go straight from the transpose PSUM tile and
            are issued on the (otherwise idle) GpSimd sequencer, which
            also hosts the collective trigger -- so each 64-token half
            ships as soon as its transpose lands and the trigger fires
            with minimal cross-engine latency."""
            osb = osbp.tile([P, 4, CD], fp, tag="osb",
                            name=f"osb_{c}_{b}")
            for hh in range(2):
                rec4 = work.tile([P, 4, 1], f32, tag="rec",
                                 name=f"rec_{c}_{b}_{hh}")
                nc.vector.reciprocal(rec4[:], pso[hh][:, :, DH:DH + 1])
                for s in range(4):
                    if (c, b) == (3, 1):
                        # last finish is collective-critical: normalize
                        # on ScalarE (idle after the final exp), DVE only
                        # does the two reciprocals
                        nc.scalar.activation(
                            osb[:, s, hh * DH:(hh + 1) * DH],
                            pso[hh][:, s, 0:DH],
                            AF.Copy, scale=rec4[:, s, :])
                    else:
                        nc.vector.tensor_scalar_mul(
                            osb[:, s, hh * DH:(hh + 1) * DH],
                            pso[hh][:, s, 0:DH], rec4[:, s, :])
            # local PE transpose: the AllToAll carries oT so the
            # projection needs no DMA transposes
            obT = work.tile([P, 4, P], fp, tag="obT", name=f"obT_{c}_{b}")
            for s in range(4):
                trp = psum.tile([P, P], fp, tag="o",
                                name=f"trp_{c}_{b}_{s}")
                nc.tensor.transpose(trp[:], osb[:, s, :], ident_sb[:])
                nc.vector.tensor_copy(obT[:, s, :], trp[:])
            # ONE staging DMA per (c, b): SWDGE descriptor generation is
            # ~1us per dma_start, so four per-subtile DMAs would delay
            # the collective trigger by ~3us.  64-token half j of
            # q-subtile s goes to dest core 2s+j.
            dst = a2ain3[b][:] if c == NCH - 1 else a2ain[c][:, b]
            nc.gpsimd.dma_start(
                dst.rearrange("j p t -> p j t"),
                obT[:].rearrange("p s (j t) -> p (s j) t", j=2))

        def proj_tile(g, b):
            return otkp.tile([P, KD, 2, 64], fp, tag="oTk",
                             name=f"oTk_{g}_{b}")

        def proj_load(oTk, g, b, hf):
            """DMA one source chunk's 8 oT contraction tiles (all in one
            transfer -- dma_start costs ~0.6us of sequencer time)."""
            c = 2 * g + hf
            src = a2aout3[b][:] if c == NCH - 1 else a2aout[c][:, b]
            nc.sync.dma_start(
                oTk[:, :, hf, :], src.rearrange("k p t -> p k t"))

        def proj_mms(oTk, g, b):
            """Projection matmuls for my 128-token slice of chunk-pair
            g, batch b against the full Wp.  PSUM->SBUF casts run on
            ScalarE (idle after the last exp) to keep DVE free."""
            psp = psum.tile([P, D], f32, tag="big", bufs=3,
                            name=f"psp_{g}_{b}")
            outsb = work.tile([P, D], fp, tag="outsb",
                              name=f"outsb_{g}_{b}")
            for hf in range(2):
                for k in range(KD):
                    nc.tensor.matmul(
                        psp[:, hf * 512:(hf + 1) * 512],
                        oTk[:, k, :, :],
                        wp_sb[:, k, hf * 512:(hf + 1) * 512],
                        start=(k == 0), stop=(k == KD - 1))
                nc.scalar.copy(
                    outsb[:, hf * 512:(hf + 1) * 512],
                    psp[:, hf * 512:(hf + 1) * 512])
            return outsb

        def emit_a2a(c):
            nc.gpsimd.collective_compute(
                "AllToAll", bass.mybir.AluOpType.bypass,
                replica_groups=RG, ins=[a2ain[c]], outs=[a2aout[c]])

        def emit_a2a3(b):
            nc.gpsimd.collective_compute(
                "AllToAll", bass.mybir.AluOpType.bypass,
                replica_groups=RG, ins=[a2ain3[b]], outs=[a2aout3[b]])

        # pipeline: attention(c) paces ScalarE; qkv(c+1) is drip-fed INTO
        # the attention k-loop so the PE fills ScalarE-paced slack.
        # Chunk collectives fire as soon as their oT is staged; chunk 3
        # goes per batch so its batch-0 collective overlaps batch 1's
        # attention and only the final 128KB collective is exposed.
        pending = []   # (chunk, batch, closure) drip units

        def filler():
            if pending:
                pending.pop(0)[2]()
            if len(pending) > 8:
                pending.pop(0)[2]()

        def drain_for(c, b):
            # emit every unit attention(c, b) depends on (its own chunk's
            # batch-b projections); later units keep dripping
            while any(t == c and bb == b for t, bb, _ in pending):
                pending.pop(0)[2]()

        # only batch 0's projections block the first scores
        units0 = qkv_units(0)
        for _, _, u in units0[:6]:
            u()
        pending.extend(units0[6:])
        for c in range(NCH):
            pending.extend(qkv_units(c + 1) if c + 1 < NCH else [])
            for b in range(B):
                drain_for(c, b)
                pso = attention_batch(c, b, filler=filler)
                finish_batch(c, b, pso)
                if b == B - 1 and c < NCH - 1:
                    emit_a2a(c)
                elif (c, b) == (NCH - 1, 0):
                    emit_a2a3(0)
        while pending:
            pending.pop(0)[2]()
        # Tail: group-0 projections fill the attention wind-down;
        # proj(1,0) runs as soon as c3b0 lands (hidden under c3b1's
        # flight); LDW warm-keepers bridge the PE across the c3b1 wait
        # so the final projection runs at full clock.  a2aout3[1] reads
        # must be emitted after emit_a2a3(1) (its only writer).
        t00, t01 = proj_tile(0, 0), proj_tile(0, 1)
        for hf in range(2):
            proj_load(t00, 0, 0, hf)
            proj_load(t01, 0, 1, hf)
        o00 = proj_mms(t00, 0, 0)
        o01 = proj_mms(t01, 0, 1)
        t10 = proj_tile(1, 0)
        proj_load(t10, 1, 0, 0)
        proj_load(t10, 1, 0, 1)
        nc.sync.dma_start(out[0, 0], o00[:])
        nc.sync.dma_start(out[0, 1], o01[:])
        o10 = proj_mms(t10, 1, 0)
        t11 = proj_tile(1, 1)
        proj_load(t11, 1, 1, 0)
        emit_a2a3(1)
        nc.sync.dma_start(out[1, 0], o10[:])
        proj_load(t11, 1, 1, 1)
        o11 = proj_mms(t11, 1, 1)
        nc.sync.dma_start(out[1, 1], o11[:])

    nc.finalize()
    return nc


def _get_nc():
    if "nc" not in _CACHE:
        _CACHE["nc"] = _build_nc()
    return _CACHE["nc"]


def kernel(x, Wq, bq, Wk, bk, Wv, bv, Wp, bp):
    global LAST_RESULT
    from concourse.bass_utils import run_bass_kernel_spmd

    x = np.asarray(x, dtype=np.float32)
    Wq = np.asarray(Wq, dtype=np.float32)
    Wk = np.asarray(Wk, dtype=np.float32)
    Wv = np.asarray(Wv, dtype=np.float32)
    Wp = np.asarray(Wp, dtype=np.float32)
    bq = np.asarray(bq, dtype=np.float32)
    bv = np.asarray(bv, dtype=np.float32)
    bp = np.asarray(bp, dtype=np.float32)

    s = DH ** -0.5
    maskf = np.where(
        np.arange(P)[:, None] <= np.arange(P)[None, :], 1.0, 0.0
    ).astype(np.float16)
    ident = np.eye(P, dtype=np.float16)
    xTg = np.ascontiguousarray(np.stack([x[0].T, x[1].T], axis=1)
                               ).astype(np.float16)
    wp16 = np.ascontiguousarray(Wp).astype(np.float16)

    in_maps = []
    for r in range(NCORES):
        cols = slice(r * CD, (r + 1) * CD)
        in_maps.append({
            "xT": xTg,
            "wq": (Wq[:, cols] * s).astype(np.float16),
            "wk": np.ascontiguousarray(Wk[:, cols]).astype(np.float16),
            "wv": np.ascontiguousarray(Wv[:, cols]).astype(np.float16),
            "wp": wp16,
            "bqp": np.ascontiguousarray((bq[cols] * s).reshape(P, 1)),
            "maskf": maskf,
            "ident": ident,
        })

    nc = _get_nc()
    res = run_bass_kernel_spmd(
        nc, in_maps, core_ids=list(range(NCORES)),
        trace=bool(int(os.environ.get("KERNEL_TRACE", "0"))))
    LAST_RESULT = res

    out = np.empty((B, T, D), dtype=np.float32)
    for r in range(NCORES):
        o = res.results[r]["out"]          # [NG, B, 128, D] fp16
        for g in range(NG):
            for hf in range(2):            # chunk 2g+hf's 64-token slice
                t0 = (2 * g + hf) * 512 + r * 64
                out[:, t0:t0 + 64, :] = \
                    o[g, :, hf * 64:(hf + 1) * 64, :].astype(np.float32)
    # bias terms that are constant w.r.t. the data path:
    #   v-bias passes through softmax rows (sum=1) -> + bv@Wp; plus bp.
    #   (bk shifts every logit in a row equally -> cancels in softmax.)
    out += (bv @ Wp + bp)[None, None, :]
    return out
